# revision 1
# baseline (speedup 1.0000x reference)
"""Causal MHA (shared q_linear) Bass kernel for 8 TRN2 NeuronCores.

Sharding: core c handles batch b=c//2, head-group g=c%2 (8 of 16 heads,
columns 512g:512g+512 of the shared projection).  Each core computes a
partial output (its head-group's contribution through Wo); the host sums
the two partials per batch and adds bo.

Compute layout (per core, S=2048 tokens, D=1024, 8 heads of hd=64):
  xT  = transpose(x) via PE               [1024, 2048]  (fp32, exact)
  qT/kT = Wq_g^T @ xT (+bq)               [512, 2048]   (fp32r matmuls)
  v   = x @ Wq_g (+bq), stored [tok, head, 65] with a fused ones column
  scoresT[k,q] = kh @ qh^T (per head, K=64, two heads packed in PE rows)
  exp on ACT with scale=1/8, additive -1e10 causal mask on PSUM
  attnT[hd+1, q] = [vh|1]^T @ expT  accumulated over k in PSUM
     row 64 = sum(exp) -> reciprocal -> rank-1 PE broadcast -> normalize
  out = attnT^T @ Wo_g  (partial, host adds the two head-groups + bo)
"""

import sys

sys.path.insert(0, "/opt/trn_rl_repo")

import numpy as np
import concourse.bass as bass  # noqa: F401
import concourse.tile as tile
from concourse import bacc, mybir
from concourse.bass_utils import run_bass_kernel_spmd

F32 = mybir.dt.float32
F32R = mybir.dt.float32r
BF16 = mybir.dt.bfloat16
AF = mybir.ActivationFunctionType

S = 2048          # tokens
D = 1024          # model dim
DL = 512          # local (per-core) projection columns = 8 heads * 64
HD = 64           # head dim
NHL = 8           # local heads
TB = 4            # token blocks of 512
JD = 8            # Din blocks of 128
NEG = -1.0e10


def build(repeat: int = 1, mode: str = "full", variant: str = "v4"):
    nc = bacc.Bacc("TRN2", target_bir_lowering=False, debug=False)
    xdt = BF16 if variant == "v7" else F32
    x_aps = {
        n: nc.dram_tensor(n, [S, D], xdt, kind="ExternalInput").ap()
        for n in ("x_q", "x_k", "x_v")
    }
    wq_ap = nc.dram_tensor("wq", [D, DL], F32, kind="ExternalInput").ap()
    bq_ap = nc.dram_tensor("bq", [DL], F32, kind="ExternalInput").ap()
    wo_ap = nc.dram_tensor("wo", [DL, D], F32, kind="ExternalInput").ap()
    tri_ap = nc.dram_tensor("tri", [128, 128], F32, kind="ExternalInput").ap()
    tri01_ap = nc.dram_tensor("tri01", [128, 128], F32, kind="ExternalInput").ap()
    id_ap = nc.dram_tensor("ident", [128, 128], F32, kind="ExternalInput").ap()
    out_ap = nc.dram_tensor("out", [S, D], F32, kind="ExternalOutput").ap()

    with tile.TileContext(nc) as tc:
        with tc.tile_pool(name="const", bufs=1) as const, \
             tc.tile_pool(name="persist", bufs=1) as persist, \
                          tc.tile_pool(name="xn", bufs=3) as xnp, \
             tc.tile_pool(name="xt", bufs=1) as xtp, \
             tc.tile_pool(name="qt", bufs=4) as qtp, \
             tc.tile_pool(name="exp", bufs=(2 if variant == "v9" else 3)) as ep, \
             tc.tile_pool(name="at", bufs=2) as atp, \
             tc.tile_pool(name="norm", bufs=1) as normp, \
             tc.tile_pool(name="ob", bufs=2) as obp, \
             tc.tile_pool(name="psS", bufs=(2 if variant == "v8" else 3), space="PSUM") as psS, \
             tc.tile_pool(name="psAcc", bufs=(3 if variant == "v8" else 2), space="PSUM") as psAcc:

            # ---- constants ----
            ident = const.tile([128, 128], F32)
            nc.sync.dma_start(ident[:], id_ap[:])
            tri = const.tile([128, 128], F32)
            nc.sync.dma_start(tri[:], tri_ap[:])
            tri01 = const.tile([128, 128], F32)
            nc.sync.dma_start(tri01[:], tri01_ap[:])
            bq_sb = const.tile([128, 4], F32)
            nc.sync.dma_start(bq_sb[:], bq_ap.rearrange("(t p) -> p t", p=128))
            bq_row = const.tile([1, DL], F32)
            nc.sync.dma_start(bq_row[:], bq_ap.rearrange("(a n) -> a n", a=1))
            bq_row_r = const.tile([1, DL], F32R)
            nc.vector.tensor_copy(bq_row_r[:], bq_row[:])
            ones_f = const.tile([128, 128], F32)
            nc.vector.memset(ones_f[:], 1.0)
            ones_r = const.tile([128, 128], F32R)
            nc.vector.tensor_copy(ones_r[:], ones_f[:])
            if variant == "v7":
                ones_b = const.tile([128, 128], BF16)
                nc.vector.tensor_copy(ones_b[:], ones_f[:])
                bq_row_b = const.tile([1, DL], BF16)
                nc.vector.tensor_copy(bq_row_b[:], bq_row[:])

            # ---- weights (cast to fp32r once) ----
            wq_r = persist.tile([128, JD, DL], BF16 if variant == "v7" else F32R)
            for j in range(JD):
                st = obp.tile([128, D], F32, tag="ob")
                nc.sync.dma_start(st[:, 0:DL], wq_ap[j * 128:(j + 1) * 128, :])
                nc.vector.tensor_copy(wq_r[:, j, :], st[:, 0:DL])
            wo_r = persist.tile([128, 4, D], F32R)
            for kt in range(4):
                st = obp.tile([128, D], F32, tag="ob")
                nc.sync.dma_start(st[:], wo_ap[kt * 128:(kt + 1) * 128, :])
                nc.vector.tensor_copy(wo_r[:, kt, :], st[:])

            # persistent per-token-block tensors
            kT = [persist.tile([128, 4, 512], F32R, name=f"kT{i}", tag=f"kT{i}") for i in range(TB)]
            vv = [persist.tile([128, 4, NHL, HD + 1], F32R, name=f"vv{i}", tag=f"vv{i}") for i in range(TB)]

            q_tiles = [None] * TB

            def _phase1_transpose(x_ap, xT, tb):
                for sub in range(4):
                    r0 = tb * 512 + sub * 128
                    # two half-tiles so transposes of D-cols 0:512 start as
                    # soon as the first 256KB lands (whole-tile dep otherwise
                    # stalls PE ~2.7us at every input boundary)
                    xh = []
                    for half in range(2):
                        xn = xnp.tile([128, DL], F32, tag=f"xn{half}")
                        nc.sync.dma_start(
                            xn[:], x_ap[r0:r0 + 128, half * DL:(half + 1) * DL]
                        )
                        xh.append(xn)
                    if variant not in ("v5", "v6"):
                        for jg in range(2):
                            pt = psS.tile([128, 512], F32, tag="sc")
                            for ji in range(4):
                                j = jg * 4 + ji
                                nc.tensor.transpose(
                                    pt[:, ji * 128:(ji + 1) * 128],
                                    xh[jg][:, ji * 128:(ji + 1) * 128],
                                    ident[:],
                                )
                            dst = xT[:, jg * 4:(jg + 1) * 4,
                                     sub * 128:(sub + 1) * 128]
                            srcv = pt[:].rearrange("p (j t) -> p j t", j=4)
                            if jg == 0 or variant == "v3":
                                nc.vector.tensor_copy(dst, srcv)
                            else:
                                nc.scalar.activation(dst, srcv, AF.Identity)
                    else:
                        pt = psS.tile([128, 2, 512], F32, tag="sc")
                        for j in range(JD):
                            nc.tensor.transpose(
                                pt[:, j // 4, (j % 4) * 128:(j % 4 + 1) * 128],
                                xn[:, j * 128:(j + 1) * 128],
                                ident[:],
                            )
                        dst = xT[:, :, sub * 128:(sub + 1) * 128]
                        srcv = pt[:].rearrange("p b (g t) -> p (b g) t", g=4)
                        if sub % 2 == 0:
                            nc.vector.tensor_copy(dst, srcv)
                        else:
                            nc.scalar.activation(dst, srcv, AF.Identity)

            def phase1(tb, rep):
                """transpose + project q,k,v for token block tb (512 tokens)."""
                for name in ("x_k", "x_v", "x_q"):
                    x_ap = x_aps[name]
                    if variant == "v7":
                        xT = xtp.tile([128, JD, 512], BF16, tag="xt")
                        for j in range(JD):
                            nc.scalar.dma_start(
                                out=xT[:, j, :],
                                in_=x_ap[tb * 512:(tb + 1) * 512,
                                         j * 128:(j + 1) * 128],
                                transpose=True,
                            )
                    else:
                        xT = xtp.tile([128, JD, 512], F32R, tag="xt")
                        _phase1_transpose(x_ap, xT, tb)
                    if name == "x_v":
                        vt = vv[tb]
                        for sub in range(4):
                            pv = psS.tile([128, 512], F32, tag="sc")
                            for j in range(JD):
                                nc.tensor.matmul(
                                    pv[:],
                                    xT[:, j, sub * 128:(sub + 1) * 128],
                                    wq_r[:, j, :],
                                    start=(j == 0),
                                    stop=False,
                                )
                            nc.tensor.matmul(
                                pv[:],
                                (ones_b if variant == "v7" else ones_r)[0:1, 0:128],
                                (bq_row_b if variant == "v7" else bq_row_r)[:],
                                start=False,
                                stop=True,
                            )
                            nc.vector.tensor_copy(
                                vt[:, sub, :, 0:HD],
                                pv[:].rearrange("p (h d) -> p h d", h=NHL),
                            )
                        nc.vector.tensor_copy(
                            vt[:, :, :, HD],
                            ones_f[:, 0:32].rearrange("p (s h) -> p s h", s=4),
                        )
                    else:
                        if name == "x_q":
                            dest = qtp.tile([128, 4, 512], F32R, tag="qt")
                            q_tiles[tb] = dest
                        else:
                            dest = kT[tb]
                        for dt_ in range(4):
                            py = psS.tile([128, 512], F32, tag="sc")
                            for j in range(JD):
                                nc.tensor.matmul(
                                    py[:],
                                    wq_r[:, j, dt_ * 128:(dt_ + 1) * 128],
                                    xT[:, j, :],
                                    start=(j == 0),
                                    stop=(j == JD - 1),
                                )
                            nc.scalar.activation(
                                dest[:, dt_, :],
                                py[:],
                                AF.Identity,
                                bias=bq_sb[:, dt_:dt_ + 1],
                            )

            def attention(Q, rep):
                """attention + Wo for query block Q (512 tokens)."""
                attnT = [atp.tile([128, 512], F32R, tag=f"at{i}", name=f"attnT{i}")
                         for i in range(4)]
                qtile = q_tiles[Q]
                nj = 4 * (Q + 1)
                for hp in range(4):
                    acc0 = psAcc.tile([128, 512], F32, tag="acc")
                    acc1 = psAcc.tile([128, 512], F32, tag="acc")

                    def emit_scores(j):
                        """scoresT pair + mask + exp for k-tile j; returns exp tile."""
                        tbj, sub = j // 4, j % 4
                        qoff = max(0, j * 128 - Q * 512)
                        ps = psS.tile([128, 2, 512], F32, tag="sc", name=f"ps{j}")
                        for hi, base in ((0, 0), (1, 64)):
                            nc.tensor.matmul(
                                ps[:, hi, qoff:],
                                kT[tbj][base:base + 64, hp,
                                        sub * 128:(sub + 1) * 128],
                                qtile[base:base + 64, hp, qoff:],
                                start=True,
                                stop=True,
                            )
                        diag = j * 128 >= Q * 512
                        if diag and variant != "v11":
                            for hi in range(2):
                                nc.vector.tensor_add(
                                    ps[:, hi, qoff:qoff + 128],
                                    ps[:, hi, qoff:qoff + 128],
                                    tri[:],
                                )
                        et = ep.tile([128, 2, 512], F32R, tag="exp", name=f"et{j}")
                        nc.scalar.activation(
                            et[:, :, qoff:], ps[:, :, qoff:], AF.Exp, scale=0.125
                        )
                        if diag and variant == "v11":
                            # zero masked entries after exp, off the PE->ACT chain
                            for hi in range(2):
                                nc.vector.tensor_mul(
                                    et[:, hi, qoff:qoff + 128],
                                    et[:, hi, qoff:qoff + 128],
                                    tri01[:],
                                )
                        return et

                    def emit_attn(j, et):
                        tbj, sub = j // 4, j % 4
                        qoff = max(0, j * 128 - Q * 512)
                        for hi, acc in ((0, acc0), (1, acc1)):
                            nc.tensor.matmul(
                                acc[0:65, qoff:],
                                vv[tbj][:, sub, hp * 2 + hi, :],
                                et[:, hi, qoff:],
                                start=(j == 0),
                                stop=(j == nj - 1),
                            )

                    # software pipeline: scores/exp run up to two k-tiles
                    # ahead of the accumulating attn matmuls so the in-order
                    # PE stream never head-blocks on the ACT exp.
                    depth = {"v3": 1, "v6": 3}.get(variant, 2)
                    ets = [emit_scores(j) for j in range(min(depth, nj))]
                    for j in range(depth, nj):
                        ets.append(emit_scores(j))
                        emit_attn(j - depth, ets[j - depth])
                    for j in range(max(0, nj - depth), nj):
                        emit_attn(j, ets[j])
                    if variant in ("v9",):
                        accs_sb = []
                        for hi, acc in ((0, acc0), (1, acc1)):
                            asb = normp.tile([128, 512], F32, tag=f"asb{hi}")
                            nc.vector.tensor_copy(asb[0:65, :], acc[0:65, :])
                            accs_sb.append(asb)
                        for hi, asb in ((0, accs_sb[0]), (1, accs_sb[1])):
                            sr = normp.tile([1, 512], F32, tag="sr")
                            nc.vector.tensor_copy(sr[0:1, :], asb[64:65, :])
                            bb = normp.tile([64, 512], F32, tag="bb")
                            nc.gpsimd.partition_broadcast(bb[:], sr[0:1, :])
                            rb = normp.tile([64, 512], F32, tag="rb")
                            nc.vector.reciprocal(rb[:], bb[:])
                            nc.vector.tensor_mul(
                                attnT[hp][hi * 64:(hi + 1) * 64, :],
                                asb[0:64, :],
                                rb[:],
                            )
                        continue_norm = False
                    else:
                        continue_norm = True
                    for hi, acc in (((0, acc0), (1, acc1)) if continue_norm else ()):
                        if variant == "v3":
                            sr = normp.tile([128, 512], F32R, tag="srr")
                            nc.vector.tensor_copy(sr[64:65, :], acc[64:65, :])
                            pb = psS.tile([128, 512], F32, tag="sc")
                            nc.tensor.matmul(
                                pb[0:64, :], ones_r[64:65, 0:64], sr[64:65, :],
                                start=True, stop=True,
                            )
                            rb = normp.tile([64, 512], F32, tag="rb")
                            nc.vector.reciprocal(rb[:], pb[0:64, :])
                        else:
                            # sum row -> DMA partition-broadcast -> wide
                            # reciprocal -> normalize (no PE/ACT involvement)
                            sr = normp.tile([1, 512], F32, tag="sr")
                            nc.vector.tensor_copy(sr[0:1, :], acc[64:65, :])
                            bb = normp.tile([64, 512], F32, tag="bb")
                            nc.gpsimd.partition_broadcast(bb[:], sr[0:1, :])
                            rb = normp.tile([64, 512], F32, tag="rb")
                            nc.vector.reciprocal(rb[:], bb[:])
                        nc.vector.tensor_mul(
                            attnT[hp][hi * 64:(hi + 1) * 64, :],
                            acc[0:64, :],
                            rb[:],
                        )
                # Wo projection for this token block
                for st_ in range(4):
                    ob = obp.tile([128, D], F32, tag="ob")
                    for nh in range(2):
                        po = psS.tile([128, 512], F32, tag="sc")
                        for kt in range(4):
                            nc.tensor.matmul(
                                po[:],
                                attnT[kt][:, st_ * 128:(st_ + 1) * 128],
                                wo_r[:, kt, nh * 512:(nh + 1) * 512],
                                start=(kt == 0),
                                stop=(kt == 3),
                            )
                        nc.vector.tensor_copy(ob[:, nh * 512:(nh + 1) * 512], po[:])
                    r0 = Q * 512 + st_ * 128
                    nc.sync.dma_start(out_ap[r0:r0 + 128, :], ob[:])

            if mode == "full":
                for rep in range(repeat):
                    if variant == "v10":
                        phase1(0, rep)
                        phase1(1, rep)
                        attention(0, rep)
                        phase1(2, rep)
                        attention(1, rep)
                        phase1(3, rep)
                        attention(2, rep)
                        attention(3, rep)
                    else:
                        for tb in range(TB):
                            phase1(tb, rep)
                        for Q in range(TB):
                            if Q == 0 and variant == "v12":
                                with tc.high_priority():
                                    attention(Q, rep)
                            else:
                                attention(Q, rep)
            elif mode == "p1":
                for rep in range(repeat):
                    for tb in range(TB):
                        phase1(tb, rep)
                for Q in range(TB):
                    attention(Q, 0)
            elif mode == "attn":
                for tb in range(TB):
                    phase1(tb, 0)
                for rep in range(repeat):
                    for Q in range(TB):
                        attention(Q, rep)

    nc.compile()
    return nc


_BUILD_CACHE = {}


def _get(repeat=1, mode="full", variant="v4"):
    key = (repeat, mode, variant)
    if key not in _BUILD_CACHE:
        _BUILD_CACHE[key] = build(repeat, mode, variant)
    return _BUILD_CACHE[key]


def make_in_maps(q, k, v, Wq, bq, Wo, bo, variant="v4"):
    import ml_dtypes
    xdt = ml_dtypes.bfloat16 if variant == "v7" else np.float32
    tri = np.where(
        np.arange(128)[:, None] <= np.arange(128)[None, :], 0.0, NEG
    ).astype(np.float32)
    tri01 = (tri == 0.0).astype(np.float32)
    ident = np.eye(128, dtype=np.float32)
    in_maps = []
    for c in range(8):
        b, g = c // 2, c % 2
        sl = slice(g * DL, (g + 1) * DL)
        in_maps.append({
            "x_q": np.ascontiguousarray(q[b]).astype(xdt),
            "x_k": np.ascontiguousarray(k[b]).astype(xdt),
            "x_v": np.ascontiguousarray(v[b]).astype(xdt),
            "wq": np.ascontiguousarray(Wq[:, sl]),
            "bq": np.ascontiguousarray(bq[sl]),
            "wo": np.ascontiguousarray(Wo[sl, :]),
            "tri": tri,
            "tri01": tri01,
            "ident": ident,
        })
    return in_maps


DEFAULT_VARIANT = "v4"


def kernel(q, k, v, Wq, bq, Wo, bo):
    q, k, v, Wq, bq, Wo, bo = (
        np.asarray(a, dtype=np.float32) for a in (q, k, v, Wq, bq, Wo, bo)
    )
    nc = _get(1, "full", DEFAULT_VARIANT)
    in_maps = make_in_maps(q, k, v, Wq, bq, Wo, bo, DEFAULT_VARIANT)
    res = run_bass_kernel_spmd(nc, in_maps, list(range(8)))
    B = q.shape[0]
    out = np.empty((B, S, D), dtype=np.float32)
    for b in range(B):
        out[b] = res.results[2 * b]["out"] + res.results[2 * b + 1]["out"] + bo
    return out

